# revision 22
# baseline (speedup 1.0000x reference)
"""MoE FFN (8 experts, top-2) Trainium2 kernel.

Strategy (expert-parallel, per sharding hint):
  - Host computes the gate (softmax + top-2 + renormalize) in float64 and
    routes tokens: core e receives the tokens whose top-2 includes expert e,
    padded to a common N_pad, transposed to [d_model, N_pad].
  - Each core runs the dense expert FFN for its expert:
        H^T = silu(W1^T x^T) * (W3^T x^T)
        y^T = W2^T H^T
    in one Bass/Tile program; H^T is staged through DRAM in bf16.
  - Host gathers y per expert and combines with the gate weights.

silu(g) is computed as g*sigmoid(g) (ACT sigmoid + DVE muls).
All matmul operands are bf16 (fp32 accumulation in PSUM): bf16 weight
loads get FWL (2x) so LDWEIGHTS hides fully under the matmuls.
"""

import math
from contextlib import ExitStack

import ml_dtypes
import numpy as np

P = 128
D_MODEL = 2048
HIDDEN = 5632
N_EXPERTS = 8
TOP_K = 2
N_CORES = 8

H_SPAN = 512    # phase-1 weight streaming span along hidden dim
D_SPAN = 512    # phase-2 resident W2 span along d_model
DT_SWEEP = 4    # d-tiles accumulated concurrently in phase 2 (PSUM banks used = 2*DT_SWEEP)

_prog_cache: dict[int, object] = {}


def _chunk_list(n_pad: int, max_chunk: int = 512) -> list[tuple[int, int]]:
    """Split [0, n_pad) into near-equal chunks <= max_chunk, multiples of 8."""
    assert n_pad % 8 == 0
    k = math.ceil(n_pad / max_chunk)
    base = (n_pad // k) // 8 * 8
    sizes = [base] * k
    extra = n_pad - base * k
    i = 0
    while extra > 0:
        sizes[i] += 8
        extra -= 8
        i = (i + 1) % k
    out = []
    n0 = 0
    for s in sizes:
        out.append((n0, s))
        n0 += s
    return out


def _build_program(d_model: int, hidden: int, n_pad: int):
    import concourse.bacc as bacc
    import concourse.mybir as mybir
    import concourse.tile as tile

    f32 = mybir.dt.float32
    bf16 = mybir.dt.bfloat16
    Sigmoid = mybir.ActivationFunctionType.Sigmoid
    mult = mybir.AluOpType.mult

    DC = d_model // P      # d chunks (contraction tiles for phase 1)
    HC = hidden // P       # h chunks
    chunks = _chunk_list(n_pad)

    nc = bacc.Bacc(
        "TRN2",
        target_bir_lowering=False,
        debug=False,
        enable_asserts=False,
        num_devices=N_CORES,
    )
    xT = nc.dram_tensor("xT", [d_model, n_pad], bf16, kind="ExternalInput").ap()
    w1 = nc.dram_tensor("w1", [d_model, hidden], bf16, kind="ExternalInput").ap()
    w3 = nc.dram_tensor("w3", [d_model, hidden], bf16, kind="ExternalInput").ap()
    w2 = nc.dram_tensor("w2", [hidden, d_model], bf16, kind="ExternalInput").ap()
    hbuf = nc.dram_tensor("hbuf", [hidden, n_pad], bf16).ap()
    yT = nc.dram_tensor("yT", [d_model, n_pad], f32, kind="ExternalOutput").ap()

    with tile.TileContext(nc) as tc, ExitStack() as ctx:
        # W2 pool opened before phase-1 pools: its SBUF range does not overlap
        # theirs, so the first W2 group's DMAs can overlap phase-1 compute.
        w2pool = ctx.enter_context(tc.tile_pool(name="w2p", bufs=1))
        # ---- phase 1: H^T = silu(W1^T x^T) * (W3^T x^T), streamed to DRAM (bf16)
        with ExitStack() as p1:
            # resident x^T, one tile per 128-row d-chunk (fine-grained deps);
            # loaded via gpsimd (SWDGE) so it doesn't queue behind weight DMAs
            xpool = p1.enter_context(tc.tile_pool(name="xp", bufs=1))
            xts = []
            for c in range(DC):
                t = xpool.tile([P, n_pad], bf16, tag=f"x{c}", name=f"x{c}")
                nc.gpsimd.dma_start(out=t[:], in_=xT[c * P : (c + 1) * P, :])
                xts.append(t)

            def xslice(c, n0, sz):
                return xts[c][:, n0 : n0 + sz]
            wpool = p1.enter_context(tc.tile_pool(name="w13", bufs=2))
            pspool = p1.enter_context(tc.tile_pool(name="ps1", bufs=1, space="PSUM"))
            spool = p1.enter_context(tc.tile_pool(name="sg", bufs=4))
            hpool = p1.enter_context(tc.tile_pool(name="hout", bufs=3))

            # PE warm-up: ~72 junk matmuls on a memset tile during the initial
            # DMA wait so HAM un-throttles (K=8/8) before the first real MM.
            # The result lands in hbuf[0:P, 0:64], which the real h-row 0
            # write fully overwrites (WAW keeps it ordered and live).
            wsrc = spool.tile([P, P], bf16, tag="warm_src", name="wsrc")
            nc.vector.memset(wsrc[:], 0.0)
            wps = pspool.tile([P, chunks[0][1]], f32, tag="pg0", bufs=2, name="warm_ps")
            for i in range(72):
                nc.tensor.matmul(
                    wps[:, :64], wsrc[:], wsrc[:, :64],
                    start=(i == 0), stop=(i == 71),
                )
            wsb = spool.tile([P, 64], bf16, tag="warm_sb", name="wsb")
            nc.scalar.copy(wsb[:], wps[:, :64])
            nc.sync.dma_start(out=hbuf[0:P, 0:64], in_=wsb[:])

            HG = hidden // H_SPAN
            HL = H_SPAN // P
            for g in range(HG):
                h0 = g * H_SPAN
                w1g = wpool.tile([P, DC * H_SPAN], bf16, tag="w1g", name="w1g")
                w3g = wpool.tile([P, DC * H_SPAN], bf16, tag="w3g", name="w3g")
                for c in range(DC):
                    nc.sync.dma_start(
                        out=w1g[:, c * H_SPAN : (c + 1) * H_SPAN],
                        in_=w1[c * P : (c + 1) * P, h0 : h0 + H_SPAN],
                    )
                for c in range(DC):
                    nc.sync.dma_start(
                        out=w3g[:, c * H_SPAN : (c + 1) * H_SPAN],
                        in_=w3[c * P : (c + 1) * P, h0 : h0 + H_SPAN],
                    )
                for hl in range(HL):
                    hrow = g * HL + hl
                    htile = hpool.tile([P, n_pad], bf16, tag="ht", name="ht")
                    pgs = [
                        pspool.tile(
                            [P, sz], f32, tag=f"pg{j}",
                            bufs=(2 if j == 0 else 1), name=f"pg{j}",
                        )
                        for j, (n0, sz) in enumerate(chunks)
                    ]
                    pvs = [
                        pspool.tile([P, sz], f32, tag=f"pv{j}", name=f"pv{j}")
                        for j, (n0, sz) in enumerate(chunks)
                    ]
                    for c in range(DC):
                        lhs = w1g[:, c * H_SPAN + hl * P : c * H_SPAN + hl * P + P]
                        for j, (n0, sz) in enumerate(chunks):
                            nc.tensor.matmul(
                                pgs[j][:],
                                lhs,
                                xslice(c, n0, sz),
                                start=(c == 0),
                                stop=(c == DC - 1),
                            )
                    for c in range(DC):
                        lhs = w3g[:, c * H_SPAN + hl * P : c * H_SPAN + hl * P + P]
                        for j, (n0, sz) in enumerate(chunks):
                            nc.tensor.matmul(
                                pvs[j][:],
                                lhs,
                                xslice(c, n0, sz),
                                start=(c == 0),
                                stop=(c == DC - 1),
                            )
                    for j, (n0, sz) in enumerate(chunks):
                        sg_t = spool.tile([P, sz], f32, tag="sg", name="sg_t")
                        nc.scalar.activation(sg_t[:], pgs[j][:], Sigmoid)
                        gv_t = spool.tile([P, sz], f32, tag="gv", name="gv_t")
                        nc.vector.tensor_tensor(gv_t[:], sg_t[:], pgs[j][:], op=mult)
                        nc.vector.tensor_tensor(
                            htile[:, n0 : n0 + sz], gv_t[:], pvs[j][:], op=mult
                        )
                    nc.sync.dma_start(
                        out=hbuf[hrow * P : (hrow + 1) * P, :], in_=htile[:]
                    )

        # ---- phase 2: y^T = W2^T H^T
        with ExitStack() as p2:
            hinpool = p2.enter_context(tc.tile_pool(name="hin", bufs=2))
            ps2 = p2.enter_context(tc.tile_pool(name="ps2", bufs=2, space="PSUM"))
            ypool = p2.enter_context(tc.tile_pool(name="yst", bufs=4))

            DG = d_model // D_SPAN
            DTS = D_SPAN // P
            for dg in range(DG):
                d0 = dg * D_SPAN
                w2g = []
                for h in range(HC):
                    t = w2pool.tile([P, D_SPAN], bf16, tag=f"w2_{h}", name=f"w2_{h}")
                    nc.sync.dma_start(
                        out=t[:], in_=w2[h * P : (h + 1) * P, d0 : d0 + D_SPAN]
                    )
                    w2g.append(t)
                for n0, sz in chunks:
                    # H strip for this n-chunk, resident across both half-sweeps
                    hstrip = hinpool.tile([P, HC * sz], bf16, tag="hs", name="hs")
                    for h in range(HC):
                        nc.sync.dma_start(
                            out=hstrip[:, h * sz : (h + 1) * sz],
                            in_=hbuf[h * P : (h + 1) * P, n0 : n0 + sz],
                        )
                    for half in range(DTS // DT_SWEEP):
                        ps = [
                            ps2.tile([P, sz], f32, tag=f"yp{q}", name=f"yp{q}")
                            for q in range(DT_SWEEP)
                        ]
                        for h in range(HC):
                            for q in range(DT_SWEEP):
                                dt = half * DT_SWEEP + q
                                nc.tensor.matmul(
                                    ps[q][:],
                                    w2g[h][:, dt * P : (dt + 1) * P],
                                    hstrip[:, h * sz : (h + 1) * sz],
                                    start=(h == 0),
                                    stop=(h == HC - 1),
                                )
                        for q in range(DT_SWEEP):
                            dt = half * DT_SWEEP + q
                            yst = ypool.tile([P, sz], f32, tag="yst", name="yst")
                            nc.scalar.copy(yst[:], ps[q][:])
                            nc.sync.dma_start(
                                out=yT[d0 + dt * P : d0 + (dt + 1) * P, n0 : n0 + sz],
                                in_=yst[:],
                            )

    nc.compile()
    return nc


def _get_program(n_pad: int):
    if n_pad not in _prog_cache:
        _prog_cache[n_pad] = _build_program(D_MODEL, HIDDEN, n_pad)
    return _prog_cache[n_pad]


def _route(x2d: np.ndarray, Wg: np.ndarray):
    """Host gate: float64 softmax + top-2 + renormalize."""
    logits = x2d.astype(np.float64) @ Wg.astype(np.float64)
    logits -= logits.max(axis=-1, keepdims=True)
    e = np.exp(logits)
    p = e / e.sum(axis=-1, keepdims=True)
    top = np.argsort(-p, axis=-1, kind="stable")[:, :TOP_K]
    w = np.take_along_axis(p, top, axis=-1)
    w = w / w.sum(axis=-1, keepdims=True)
    return top, w.astype(np.float32)


def _prepare(inputs: dict):
    x = np.asarray(inputs["x"], dtype=np.float32)
    Wg = np.asarray(inputs["Wg"], dtype=np.float32)
    W1 = np.asarray(inputs["W1"], dtype=np.float32)
    W3 = np.asarray(inputs["W3"], dtype=np.float32)
    W2 = np.asarray(inputs["W2"], dtype=np.float32)

    b, s, d = x.shape
    T = b * s
    x2d = np.ascontiguousarray(x.reshape(T, d))

    top, wts = _route(x2d, Wg)

    tok_lists = []
    wt_lists = []
    for e in range(N_EXPERTS):
        mask = top == e  # [T, K]
        toks = np.where(mask.any(axis=-1))[0]
        we = wts[toks][mask[toks]]  # one weight per selected token
        tok_lists.append(toks)
        wt_lists.append(we.astype(np.float32))

    max_count = max(len(t) for t in tok_lists)
    n_pad = max(((max_count + 7) // 8) * 8, 24)

    nc = _get_program(n_pad)

    W1bf = W1.astype(ml_dtypes.bfloat16)
    W3bf = W3.astype(ml_dtypes.bfloat16)
    W2bf = W2.astype(ml_dtypes.bfloat16)
    x2dbf = x2d.astype(ml_dtypes.bfloat16)
    in_maps = []
    for e in range(N_EXPERTS):
        toks = tok_lists[e]
        xTe = np.zeros((d, n_pad), dtype=ml_dtypes.bfloat16)
        xTe[:, : len(toks)] = x2dbf[toks].T
        in_maps.append(
            {
                "xT": xTe,
                "w1": np.ascontiguousarray(W1bf[e]),
                "w3": np.ascontiguousarray(W3bf[e]),
                "w2": np.ascontiguousarray(W2bf[e]),
            }
        )

    return nc, in_maps, tok_lists, wt_lists, (b, s, d)


def _combine(results, tok_lists, wt_lists, shape):
    b, s, d = shape
    out2d = np.zeros((b * s, d), dtype=np.float32)
    for e in range(N_EXPERTS):
        toks = tok_lists[e]
        yTe = results[e]["yT"]
        ye = yTe[:, : len(toks)].T  # [n_e, d]
        out2d[toks] += wt_lists[e][:, None] * ye
    return out2d.reshape(b, s, d)


def _ensure_trace_hooks():
    """If BASS_TRACE is set, run_bass_kernel_spmd imports antenv.axon_hooks,
    which some images lack. Provide the standard shim (ctypes into the axon
    .so) when missing, and make the artifact upload failure-tolerant."""
    import sys

    try:
        import antenv.axon_hooks  # noqa: F401
        return
    except ImportError:
        pass
    import contextlib
    import ctypes
    import types

    so_path = "/opt/axon/libaxon_pjrt.so"
    hook = None
    try:
        lib = ctypes.CDLL(so_path)
        lib.axon_start_nrt_profile.argtypes = [
            ctypes.POINTER(ctypes.c_int64),
            ctypes.c_size_t,
        ]
        lib.axon_start_nrt_profile.restype = ctypes.c_int64
        lib.axon_stop_nrt_profile.argtypes = [ctypes.c_char_p]
        lib.axon_stop_nrt_profile.restype = ctypes.c_int64

        @contextlib.contextmanager
        def _hook(output_dir, device_ids):
            import jax

            jax.devices()
            if device_ids:
                ids = (ctypes.c_int64 * len(device_ids))(*device_ids)
                rc = lib.axon_start_nrt_profile(ids, len(device_ids))
            else:
                rc = lib.axon_start_nrt_profile(None, 0)
            if rc != 0:
                raise RuntimeError(f"axon_start_nrt_profile rc={rc}")
            try:
                yield
            finally:
                lib.axon_stop_nrt_profile(str(output_dir).encode())

        hook = _hook
    except Exception:
        hook = None

    mod = types.ModuleType("antenv.axon_hooks")
    state = {"hook": hook}
    mod.get_axon_ntff_profile_hook = lambda: state["hook"]
    mod.set_axon_ntff_profile_hook = lambda h: state.update(hook=h)
    sys.modules["antenv.axon_hooks"] = mod
    try:
        import antenv

        antenv.axon_hooks = mod
    except ImportError:
        pass

    import concourse.bass_utils as bu

    orig_upload = bu.upload_artifacts

    def _safe_upload(tmpdir):
        try:
            return orig_upload(tmpdir)
        except Exception:
            return f"local://{tmpdir}"

    bu.upload_artifacts = _safe_upload


def kernel(**inputs) -> np.ndarray:
    from concourse.bass_utils import run_bass_kernel_spmd

    _ensure_trace_hooks()
    nc, in_maps, tok_lists, wt_lists, shape = _prepare(inputs)
    res = run_bass_kernel_spmd(nc, in_maps, core_ids=list(range(N_CORES)))
    return _combine(res.results, tok_lists, wt_lists, shape)


# revision 23
# speedup vs baseline: 1.1663x; 1.1663x over previous
"""MoE FFN (8 experts, top-2) Trainium2 kernel.

Strategy (expert-parallel, per sharding hint):
  - Host computes the gate (softmax + top-2 + renormalize) in float64 and
    routes tokens: core e receives the tokens whose top-2 includes expert e,
    padded to a common N_pad, transposed to [d_model, N_pad].
  - Each core runs the dense expert FFN for its expert:
        H^T = silu(W1^T x^T) * (W3^T x^T)
        y^T = W2^T H^T
    in one Bass/Tile program; H^T is staged through DRAM in bf16.
  - Host gathers y per expert and combines with the gate weights.

silu(g) is computed as g*sigmoid(g) (ACT sigmoid + DVE muls).
All matmul operands are bf16 (fp32 accumulation in PSUM): bf16 weight
loads get FWL (2x) so LDWEIGHTS hides fully under the matmuls.
"""

import math
from contextlib import ExitStack

import ml_dtypes
import numpy as np

P = 128
D_MODEL = 2048
HIDDEN = 5632
N_EXPERTS = 8
TOP_K = 2
N_CORES = 8

H_SPAN = 256    # phase-1 weight streaming span along hidden dim
D_SPAN = 1024   # phase-2 resident W2 span along d_model
DT_SWEEP = 4    # d-tiles accumulated concurrently in phase 2 (PSUM banks used = 2*DT_SWEEP)

_prog_cache: dict[int, object] = {}


def _chunk_list(n_pad: int, max_chunk: int = 512) -> list[tuple[int, int]]:
    """Split [0, n_pad) into near-equal chunks <= max_chunk, multiples of 8."""
    assert n_pad % 8 == 0
    k = math.ceil(n_pad / max_chunk)
    base = (n_pad // k) // 8 * 8
    sizes = [base] * k
    extra = n_pad - base * k
    i = 0
    while extra > 0:
        sizes[i] += 8
        extra -= 8
        i = (i + 1) % k
    out = []
    n0 = 0
    for s in sizes:
        out.append((n0, s))
        n0 += s
    return out


def _build_program(d_model: int, hidden: int, n_pad: int):
    import concourse.bacc as bacc
    import concourse.mybir as mybir
    import concourse.tile as tile

    f32 = mybir.dt.float32
    bf16 = mybir.dt.bfloat16
    Sigmoid = mybir.ActivationFunctionType.Sigmoid
    mult = mybir.AluOpType.mult

    DC = d_model // P      # d chunks (contraction tiles for phase 1)
    HC = hidden // P       # h chunks
    chunks = _chunk_list(n_pad)

    nc = bacc.Bacc(
        "TRN2",
        target_bir_lowering=False,
        debug=False,
        enable_asserts=False,
        num_devices=N_CORES,
    )
    xT = nc.dram_tensor("xT", [d_model, n_pad], bf16, kind="ExternalInput").ap()
    w1 = nc.dram_tensor("w1", [d_model, hidden], bf16, kind="ExternalInput").ap()
    w3 = nc.dram_tensor("w3", [d_model, hidden], bf16, kind="ExternalInput").ap()
    w2 = nc.dram_tensor("w2", [hidden, d_model], bf16, kind="ExternalInput").ap()
    hbuf = nc.dram_tensor("hbuf", [hidden, n_pad], bf16).ap()
    yT = nc.dram_tensor("yT", [d_model, n_pad], f32, kind="ExternalOutput").ap()

    with tile.TileContext(nc) as tc, ExitStack() as ctx:
        # W2 pool opened before phase-1 pools: its SBUF range does not overlap
        # theirs, so the first W2 group's DMAs can overlap phase-1 compute.
        w2pool = ctx.enter_context(tc.tile_pool(name="w2p", bufs=1))
        # ---- phase 1: H^T = silu(W1^T x^T) * (W3^T x^T), streamed to DRAM (bf16)
        with ExitStack() as p1:
            # resident x^T, one tile per 128-row d-chunk (fine-grained deps);
            # loaded via gpsimd (SWDGE) so it doesn't queue behind weight DMAs
            xpool = p1.enter_context(tc.tile_pool(name="xp", bufs=1))
            xts = []
            for c in range(DC):
                t = xpool.tile([P, n_pad], bf16, tag=f"x{c}", name=f"x{c}")
                nc.gpsimd.dma_start(out=t[:], in_=xT[c * P : (c + 1) * P, :])
                xts.append(t)

            def xslice(c, n0, sz):
                return xts[c][:, n0 : n0 + sz]
            wpool = p1.enter_context(tc.tile_pool(name="w13", bufs=2))
            pspool = p1.enter_context(tc.tile_pool(name="ps1", bufs=1, space="PSUM"))
            spool = p1.enter_context(tc.tile_pool(name="sg", bufs=4))
            hpool = p1.enter_context(tc.tile_pool(name="hout", bufs=3))

            # PE warm-up: ~72 junk matmuls on a memset tile during the initial
            # DMA wait so HAM un-throttles (K=8/8) before the first real MM.
            # The result lands in hbuf[0:P, 0:64], which the real h-row 0
            # write fully overwrites (WAW keeps it ordered and live).
            wsrc = spool.tile([P, P], bf16, tag="warm_src", name="wsrc")
            nc.vector.memset(wsrc[:], 0.0)
            wps = pspool.tile([P, chunks[0][1]], f32, tag="pg0", bufs=2, name="warm_ps")
            for i in range(72):
                nc.tensor.matmul(
                    wps[:, :64], wsrc[:], wsrc[:, :64],
                    start=(i == 0), stop=(i == 71),
                )
            wsb = spool.tile([P, 64], bf16, tag="warm_sb", name="wsb")
            nc.scalar.copy(wsb[:], wps[:, :64])
            nc.sync.dma_start(out=hbuf[0:P, 0:64], in_=wsb[:])

            HG = hidden // H_SPAN
            HL = H_SPAN // P
            for g in range(HG):
                h0 = g * H_SPAN
                w1g = wpool.tile([P, DC * H_SPAN], bf16, tag="w1g", name="w1g")
                w3g = wpool.tile([P, DC * H_SPAN], bf16, tag="w3g", name="w3g")
                for c in range(DC):
                    nc.sync.dma_start(
                        out=w1g[:, c * H_SPAN : (c + 1) * H_SPAN],
                        in_=w1[c * P : (c + 1) * P, h0 : h0 + H_SPAN],
                    )
                for c in range(DC):
                    nc.sync.dma_start(
                        out=w3g[:, c * H_SPAN : (c + 1) * H_SPAN],
                        in_=w3[c * P : (c + 1) * P, h0 : h0 + H_SPAN],
                    )
                for hl in range(HL):
                    hrow = g * HL + hl
                    htile = hpool.tile([P, n_pad], bf16, tag="ht", name="ht")
                    pgs = [
                        pspool.tile(
                            [P, sz], f32, tag=f"pg{j}",
                            bufs=(2 if j == 0 else 1), name=f"pg{j}",
                        )
                        for j, (n0, sz) in enumerate(chunks)
                    ]
                    pvs = [
                        pspool.tile([P, sz], f32, tag=f"pv{j}", name=f"pv{j}")
                        for j, (n0, sz) in enumerate(chunks)
                    ]
                    for c in range(DC):
                        lhs = w1g[:, c * H_SPAN + hl * P : c * H_SPAN + hl * P + P]
                        for j, (n0, sz) in enumerate(chunks):
                            nc.tensor.matmul(
                                pgs[j][:],
                                lhs,
                                xslice(c, n0, sz),
                                start=(c == 0),
                                stop=(c == DC - 1),
                            )
                    for c in range(DC):
                        lhs = w3g[:, c * H_SPAN + hl * P : c * H_SPAN + hl * P + P]
                        for j, (n0, sz) in enumerate(chunks):
                            nc.tensor.matmul(
                                pvs[j][:],
                                lhs,
                                xslice(c, n0, sz),
                                start=(c == 0),
                                stop=(c == DC - 1),
                            )
                    for j, (n0, sz) in enumerate(chunks):
                        sg_t = spool.tile([P, sz], f32, tag="sg", name="sg_t")
                        nc.scalar.activation(sg_t[:], pgs[j][:], Sigmoid)
                        gv_t = spool.tile([P, sz], f32, tag="gv", name="gv_t")
                        nc.vector.tensor_tensor(gv_t[:], sg_t[:], pgs[j][:], op=mult)
                        nc.vector.tensor_tensor(
                            htile[:, n0 : n0 + sz], gv_t[:], pvs[j][:], op=mult
                        )
                    nc.sync.dma_start(
                        out=hbuf[hrow * P : (hrow + 1) * P, :], in_=htile[:]
                    )

        # ---- phase 2: y^T = W2^T H^T
        with ExitStack() as p2:
            hinpool = p2.enter_context(tc.tile_pool(name="hin", bufs=2))
            ps2 = p2.enter_context(tc.tile_pool(name="ps2", bufs=2, space="PSUM"))
            ypool = p2.enter_context(tc.tile_pool(name="yst", bufs=4))

            DG = d_model // D_SPAN
            DTS = D_SPAN // P
            for dg in range(DG):
                d0 = dg * D_SPAN
                w2g = []
                for h in range(HC):
                    t = w2pool.tile([P, D_SPAN], bf16, tag=f"w2_{h}", name=f"w2_{h}")
                    nc.sync.dma_start(
                        out=t[:], in_=w2[h * P : (h + 1) * P, d0 : d0 + D_SPAN]
                    )
                    w2g.append(t)
                for n0, sz in chunks:
                    # H strip for this n-chunk, resident across both half-sweeps
                    hstrip = hinpool.tile([P, HC * sz], bf16, tag="hs", name="hs")
                    for h in range(HC):
                        nc.sync.dma_start(
                            out=hstrip[:, h * sz : (h + 1) * sz],
                            in_=hbuf[h * P : (h + 1) * P, n0 : n0 + sz],
                        )
                    for half in range(DTS // DT_SWEEP):
                        ps = [
                            ps2.tile([P, sz], f32, tag=f"yp{q}", name=f"yp{q}")
                            for q in range(DT_SWEEP)
                        ]
                        for h in range(HC):
                            for q in range(DT_SWEEP):
                                dt = half * DT_SWEEP + q
                                nc.tensor.matmul(
                                    ps[q][:],
                                    w2g[h][:, dt * P : (dt + 1) * P],
                                    hstrip[:, h * sz : (h + 1) * sz],
                                    start=(h == 0),
                                    stop=(h == HC - 1),
                                )
                        for q in range(DT_SWEEP):
                            dt = half * DT_SWEEP + q
                            yst = ypool.tile([P, sz], f32, tag="yst", name="yst")
                            nc.scalar.copy(yst[:], ps[q][:])
                            nc.sync.dma_start(
                                out=yT[d0 + dt * P : d0 + (dt + 1) * P, n0 : n0 + sz],
                                in_=yst[:],
                            )

    nc.compile()
    return nc


def _get_program(n_pad: int):
    if n_pad not in _prog_cache:
        _prog_cache[n_pad] = _build_program(D_MODEL, HIDDEN, n_pad)
    return _prog_cache[n_pad]


def _route(x2d: np.ndarray, Wg: np.ndarray):
    """Host gate: float64 softmax + top-2 + renormalize."""
    logits = x2d.astype(np.float64) @ Wg.astype(np.float64)
    logits -= logits.max(axis=-1, keepdims=True)
    e = np.exp(logits)
    p = e / e.sum(axis=-1, keepdims=True)
    top = np.argsort(-p, axis=-1, kind="stable")[:, :TOP_K]
    w = np.take_along_axis(p, top, axis=-1)
    w = w / w.sum(axis=-1, keepdims=True)
    return top, w.astype(np.float32)


def _prepare(inputs: dict):
    x = np.asarray(inputs["x"], dtype=np.float32)
    Wg = np.asarray(inputs["Wg"], dtype=np.float32)
    W1 = np.asarray(inputs["W1"], dtype=np.float32)
    W3 = np.asarray(inputs["W3"], dtype=np.float32)
    W2 = np.asarray(inputs["W2"], dtype=np.float32)

    b, s, d = x.shape
    T = b * s
    x2d = np.ascontiguousarray(x.reshape(T, d))

    top, wts = _route(x2d, Wg)

    tok_lists = []
    wt_lists = []
    for e in range(N_EXPERTS):
        mask = top == e  # [T, K]
        toks = np.where(mask.any(axis=-1))[0]
        we = wts[toks][mask[toks]]  # one weight per selected token
        tok_lists.append(toks)
        wt_lists.append(we.astype(np.float32))

    max_count = max(len(t) for t in tok_lists)
    n_pad = max(((max_count + 7) // 8) * 8, 24)

    nc = _get_program(n_pad)

    W1bf = W1.astype(ml_dtypes.bfloat16)
    W3bf = W3.astype(ml_dtypes.bfloat16)
    W2bf = W2.astype(ml_dtypes.bfloat16)
    x2dbf = x2d.astype(ml_dtypes.bfloat16)
    in_maps = []
    for e in range(N_EXPERTS):
        toks = tok_lists[e]
        xTe = np.zeros((d, n_pad), dtype=ml_dtypes.bfloat16)
        xTe[:, : len(toks)] = x2dbf[toks].T
        in_maps.append(
            {
                "xT": xTe,
                "w1": np.ascontiguousarray(W1bf[e]),
                "w3": np.ascontiguousarray(W3bf[e]),
                "w2": np.ascontiguousarray(W2bf[e]),
            }
        )

    return nc, in_maps, tok_lists, wt_lists, (b, s, d)


def _combine(results, tok_lists, wt_lists, shape):
    b, s, d = shape
    out2d = np.zeros((b * s, d), dtype=np.float32)
    for e in range(N_EXPERTS):
        toks = tok_lists[e]
        yTe = results[e]["yT"]
        ye = yTe[:, : len(toks)].T  # [n_e, d]
        out2d[toks] += wt_lists[e][:, None] * ye
    return out2d.reshape(b, s, d)


def _ensure_trace_hooks():
    """If BASS_TRACE is set, run_bass_kernel_spmd imports antenv.axon_hooks,
    which some images lack. Provide the standard shim (ctypes into the axon
    .so) when missing, and make the artifact upload failure-tolerant."""
    import sys

    try:
        import antenv.axon_hooks  # noqa: F401
        return
    except ImportError:
        pass
    import contextlib
    import ctypes
    import types

    so_path = "/opt/axon/libaxon_pjrt.so"
    hook = None
    try:
        lib = ctypes.CDLL(so_path)
        lib.axon_start_nrt_profile.argtypes = [
            ctypes.POINTER(ctypes.c_int64),
            ctypes.c_size_t,
        ]
        lib.axon_start_nrt_profile.restype = ctypes.c_int64
        lib.axon_stop_nrt_profile.argtypes = [ctypes.c_char_p]
        lib.axon_stop_nrt_profile.restype = ctypes.c_int64

        @contextlib.contextmanager
        def _hook(output_dir, device_ids):
            import jax

            jax.devices()
            if device_ids:
                ids = (ctypes.c_int64 * len(device_ids))(*device_ids)
                rc = lib.axon_start_nrt_profile(ids, len(device_ids))
            else:
                rc = lib.axon_start_nrt_profile(None, 0)
            if rc != 0:
                raise RuntimeError(f"axon_start_nrt_profile rc={rc}")
            try:
                yield
            finally:
                lib.axon_stop_nrt_profile(str(output_dir).encode())

        hook = _hook
    except Exception:
        hook = None

    mod = types.ModuleType("antenv.axon_hooks")
    state = {"hook": hook}
    mod.get_axon_ntff_profile_hook = lambda: state["hook"]
    mod.set_axon_ntff_profile_hook = lambda h: state.update(hook=h)
    sys.modules["antenv.axon_hooks"] = mod
    try:
        import antenv

        antenv.axon_hooks = mod
    except ImportError:
        pass

    import concourse.bass_utils as bu

    orig_upload = bu.upload_artifacts

    def _safe_upload(tmpdir):
        try:
            return orig_upload(tmpdir)
        except Exception:
            return f"local://{tmpdir}"

    bu.upload_artifacts = _safe_upload


def kernel(**inputs) -> np.ndarray:
    from concourse.bass_utils import run_bass_kernel_spmd

    _ensure_trace_hooks()
    nc, in_maps, tok_lists, wt_lists, shape = _prepare(inputs)
    res = run_bass_kernel_spmd(nc, in_maps, core_ids=list(range(N_CORES)))
    return _combine(res.results, tok_lists, wt_lists, shape)
